# revision 12
# baseline (speedup 1.0000x reference)
"""DeBut 2D conv (32,128,56,56) -> (32,256,56,56) on 8 axon TRN2 NeuronCores.

The butterfly product W3@W2@W1 composes to a block-diagonal (256,1152) matrix
with 32 blocks of (8 out x 36 in). Finer than the 2-chunk split: output
GROUP g of 32 channels (32g..32g+31, g=0..7) depends only on input features
144g..144g+143, which live inside kernel positions g and g+1. So group g is
exactly TWO kpos-pure K=128 matmuls with weights W[32g:+32, 128(g+j):+128].T
(j=0,1) - the zero rows come for free from block-diagonality.

The PE array is addressed in 128x32 column-tiled mode: 4 col quadrants run
4 concurrent M=32 matmuls, each streaming its own moving span. Per strip
that is 16 matmuls in 4 concurrent rounds = 4x462 PE cycles instead of the
9x462 of the 2-chunk scheme (2.25x less PE stream time), pushing the kernel
from PE-bound to the DMA roofline.

Per-core layout (batch sharded 4 images/core):
  x arrives host-padded to 56x58 (left/right zero cols only), converted to
  bf16 (rel-err budget is 2e-2; full-bf16 pipeline lands ~4e-3), and is
  DMA'd in 4 strip-aligned cuts per image into rows 1..56 of one of 4
  resident SBUF slots; top/bottom pad rows are persistent SBUF zeros.
  bf16 halves both DMA directions; the PE streams 1 col/cycle either way.
  7 strips of 8 output rows; moving operand = contiguous 462-col span of the
  padded image; psum (128, 8, 58) f32; drain cols 0:56 + bias via DVE
  tensor_scalar_add (chunk0) / ACT activation (chunk1), casting to bf16.
  Output pieces are split across BOTH the SWDGE (gpsimd) and ACT HWDGE
  queues; inputs ride the SP HWDGE queue.
  The repeat loop (timing harness) is unrolled 8x inside tc.For_i: each
  For_i iteration ends in a full 5-engine + DMA barrier (~5-8us), so deep
  unroll amortizes it; repeat counts stay exact via trailing bodies.
"""
import numpy as np

# ---- problem constants (hardcoded; kernel.py must be self-contained) ----
B, C_IN, H, W = 32, 128, 56, 56
C_OUT = 256
KS = 3
N_CORES = 8
B_LOC = B // N_CORES          # 4 images per core
HP = H + 2                    # 58
PADDED = HP * HP + 8          # 3372, slack for junk-column overreads
STRIP_ROWS = 8
N_STRIPS = H // STRIP_ROWS    # 7
N_MM = STRIP_ROWS * HP        # 464 moving columns per matmul
N_OUT = STRIP_ROWS * W        # 448 valid columns per strip
R_SHAPES = [(768, 1152, 2, 3, 1), (512, 768, 2, 3, 2), (256, 512, 2, 4, 4)]
# (chunk, kpos) pairs: chunk0 -> kpos 0..4, chunk1 -> kpos 4..8
CHUNK_KPOS = [(0, k) for k in range(5)] + [(1, k) for k in range(4, 9)]

_RUNNER = None


def _compose_w(twiddle: np.ndarray) -> np.ndarray:
    """Compose butterfly factors into the dense (256, 1152) matrix (float64)."""
    W_full = None
    temp = 0
    for (osz, isz, row, col, diag) in R_SHAPES:
        npar = col * osz
        nb = isz // (col * diag)
        t = twiddle[temp:temp + npar].astype(np.float64)
        t = t.reshape(nb, diag, row, col).transpose(0, 2, 3, 1)  # (n, r, c, d)
        temp += npar
        Ws = np.zeros((osz, isz), np.float64)
        # out index n*row*diag + r*diag + d ; in index n*col*diag + c*diag + d
        for d in range(diag):
            for r in range(row):
                for c in range(col):
                    out_idx = np.arange(nb) * row * diag + r * diag + d
                    in_idx = np.arange(nb) * col * diag + c * diag + d
                    Ws[out_idx, in_idx] = t[:, r, c, d]
        W_full = Ws if W_full is None else Ws @ W_full
    return W_full  # (256, 1152)


def _build_nc(repeat: int = 1, trace_sim: bool = False, mode: str = 'full',
              outp_bufs: int = 3, psum_bufs: int = 6, k4merge: bool = True,
              n_xpad: int = 4, drain_split: bool = True, opt: bool = True,
              out_pool: bool = True, early_out: bool = False,
              out_dt: str = 'bf16', out_split2: bool = True,
              in_dt: str = 'bf16', in_cuts: int = 1, unroll: int = 8,
              cuts2: bool = True, tail_sync: bool = False,
              scheme: str = 'grp32'):
    import concourse.bass as bass  # noqa: F401
    from concourse import bacc
    import concourse.mybir as mybir
    from concourse.tile import TileContext

    f32 = mybir.dt.float32
    f32r = mybir.dt.float32r
    bf16 = mybir.dt.bfloat16

    nc = bacc.Bacc("TRN2", target_bir_lowering=False, debug=False,
                   num_devices=N_CORES)
    # xs/wts are declared float32r: same 4-byte layout (numpy float32 binds),
    # lets plain HWDGE DMAs feed the f32r matmuls with no cast pass.
    # xs arrives column-padded to 56x58 so the in-DMA is one contiguous
    # ~13KB run per partition; top/bottom pad rows stay resident zeros.
    idt = bf16 if in_dt == 'bf16' else f32r
    xs = nc.declare_dram_parameter("xs", [B_LOC, C_IN, H * HP], idt,
                                   isOutput=False)
    # grp32: 16 weight mats [128 in-ch, 32 outs], slot 2g+j = group g piece j
    NW, WM = (16, 32) if scheme == 'grp32' else (10, 128)
    wts = nc.declare_dram_parameter("wts", [NW, C_IN, WM], idt, isOutput=False)
    biasT = nc.declare_dram_parameter("biasT", [128, 2], f32, isOutput=False)
    # output stored bf16: halves out-DMA traffic (rel-err budget is 2e-2,
    # bf16 rounding adds ~2e-3); host upcasts to f32
    odt = bf16 if out_dt == 'bf16' else f32
    ys = nc.declare_dram_parameter("ys", [B_LOC, 2, 128, H * W], odt,
                                   isOutput=True)

    K4MERGE = k4merge
    with TileContext(nc, trace_sim=trace_sim) as tc:
        with tc.tile_pool(name="sbuf", bufs=1) as cpool, \
             tc.tile_pool(name="outp", bufs=outp_bufs) as opool, \
             tc.tile_pool(name="psum", bufs=psum_bufs, space="PSUM") as ppool:
            # persistent padded-image slots; 8-col slack zeroed once below
            N_XPAD = n_xpad
            xpads = [cpool.tile([C_IN, PADDED], idt, tag=f"xpad{i}",
                                name=f"xpad{i}")
                     for i in range(N_XPAD)]
            zrow = cpool.tile([C_IN, HP + 8], f32 if in_dt != 'bf16' else bf16,
                              tag="zrow")
            nc.vector.memset(zrow[:], 0.0)
            for xp in xpads:
                nc.vector.tensor_copy(xp[:, :HP], zrow[:, :HP])   # top pad row
                # bottom pad row + slack (never overwritten by image DMAs)
                nc.vector.tensor_copy(xp[:, (HP - 1) * HP:], zrow[:])
            # first image on the SP HWDGE queue, weights+bias on the ACT HWDGE
            # queue - they land in parallel
            # strip s reads image rows 8s-1..8s+9; boundaries (10,26,42)
            # stagger the per-strip waits as c0,c1,c1,c2,c2,c3,c3
            CUTS = ((0, 10 * HP), (10 * HP, 26 * HP), (26 * HP, 42 * HP),
                    (42 * HP, H * HP)) if cuts2 else \
                   ((0, 9 * HP), (9 * HP, 24 * HP), (24 * HP, 40 * HP),
                    (40 * HP, H * HP))
            if mode not in ('no_in', 'pe_only'):
                cuts = CUTS if opt else ((0, 34 * HP), (34 * HP, H * HP))
                for lo, hi in cuts:
                    nc.sync.dma_start(out=xpads[0][:, HP + lo:HP + hi],
                                      in_=xs[0, :, lo:hi])
            wt_r = cpool.tile([C_IN, NW, WM], idt, tag="wtr")
            # chunk0's slots land first so the first matmul group can start
            # before the full weight transfer completes
            nc.scalar.dma_start(out=wt_r[:, 0:NW // 2, :],
                                in_=wts.ap()[0:NW // 2]
                                .rearrange("i c m -> c i m"))
            nc.scalar.dma_start(out=wt_r[:, NW // 2:NW, :],
                                in_=wts.ap()[NW // 2:NW]
                                .rearrange("i c m -> c i m"))
            bias_sb = cpool.tile([128, 2], f32, tag="bias")
            nc.scalar.dma_start(out=bias_sb[:], in_=biasT.ap())
            scratch = cpool.tile([C_IN, H * HP], idt, tag="scr",
                                 name="scr") \
                if mode == 'scratch_in' else None

            # out-DMA split points; spreading pieces across the image keeps
            # the out queues busy from strip 1 on and shrinks the tail
            if early_out == 3:
                OUT_SPLITS = {s: (s * N_OUT, (s + 1) * N_OUT)
                              for s in range(N_STRIPS)}
            elif early_out == 2:
                OUT_SPLITS = {1: (0, 2 * N_OUT), 3: (2 * N_OUT, 4 * N_OUT),
                              5: (4 * N_OUT, 6 * N_OUT),
                              6: (6 * N_OUT, 7 * N_OUT)}
            elif early_out:
                OUT_SPLITS = {2: (0, 3 * N_OUT), 4: (3 * N_OUT, 5 * N_OUT),
                              5: (5 * N_OUT, 6 * N_OUT),
                              6: (6 * N_OUT, 7 * N_OUT)}
            else:
                OUT_SPLITS = {3: (0, 4 * N_OUT), 5: (4 * N_OUT, 6 * N_OUT),
                              6: (6 * N_OUT, 7 * N_OUT)}

            def body():
                for b in range(B_LOC):
                    if mode == 'scratch_in' and (b > 0 or repeat > 1):
                        # same DMA traffic, but no PE dependency: writes go to
                        # a scratch tile nobody reads (concurrency probe)
                        nc.sync.dma_start(out=scratch[:], in_=xs[b])
                    elif (b > 0 or repeat > 1) and mode not in ('no_in',
                                                                'pe_only'):
                        # in_cuts: 1 = cut every image, 2 = cut only the
                        # first (post-barrier) image — later images hide
                        # under compute, and fewer DMAs means less
                        # completion-semaphore lane aliasing
                        if in_cuts == 1 or (in_cuts == 2 and b == 0):
                            # strip-aligned pieces: first strip's matmuls wait
                            # only the first cut, not the whole image
                            for lo, hi in CUTS:
                                nc.sync.dma_start(
                                    out=xpads[b % N_XPAD][:, HP + lo:HP + hi],
                                    in_=xs[b, :, lo:hi])
                        else:
                            nc.sync.dma_start(
                                out=xpads[b % N_XPAD][:, HP:HP + H * HP],
                                in_=xs[b])

                    xp = xpads[b % N_XPAD]

                    nmm = N_MM - 2 if opt else N_MM

                    def span(s, k):
                        di, dj = k // 3, k % 3
                        st = (STRIP_ROWS * s + di) * HP + dj
                        return xp[:, st:st + nmm], st

                    out_sb = opool.tile([128, 2, H * W], odt, tag="outsb")
                    if mode == 'dma_only':
                        nc.vector.memset(out_sb[:, :, 0:4], 0.0)
                    for s in range(N_STRIPS):
                        if mode == 'dma_only':
                            pass
                        elif scheme == 'grp32':
                            # 16 col-tiled matmuls (128x32 mode): group g ->
                            # psum quadrant g%4 of chunk g//4; piece j streams
                            # the kpos-(g+j) span. Issue in 4-lane rounds so
                            # the 4 col quadrants run concurrently.
                            ps0 = ppool.tile([128, STRIP_ROWS, HP], f32,
                                             tag="ps")
                            ps1 = ppool.tile([128, STRIP_ROWS, HP], f32,
                                             tag="ps")
                            pss = [ps0, ps1]
                            pfs = [p.rearrange("p r w -> p (r w)") for p in pss]
                            for chunk in range(2):
                                for j in range(2):
                                    for q in range(4):
                                        g = chunk * 4 + q
                                        nc.tensor.matmul(
                                            pfs[chunk][32 * q:32 * q + 32,
                                                       :nmm],
                                            wt_r[:, 2 * g + j, :],
                                            span(s, g + j)[0],
                                            start=(j == 0), stop=(j == 1),
                                            tile_position=(0, 32 * q),
                                            skip_group_check=True)
                        elif not K4MERGE:
                            pss = []
                            for chunk in range(2):
                                ps = ppool.tile([128, STRIP_ROWS, HP], f32,
                                                tag="ps")
                                pss.append(ps)
                                psflat = ps.rearrange("p r w -> p (r w)")
                                for kidx in range(5):
                                    _, k = CHUNK_KPOS[chunk * 5 + kidx]
                                    nc.tensor.matmul(
                                        psflat[:, :nmm],
                                        wt_r[:, chunk * 5 + kidx, :],
                                        span(s, k)[0],
                                        start=(kidx == 0), stop=(kidx == 4))
                        else:
                            # chunk0 <- k0..3, chunk1 <- k5..8 (full-array MMs),
                            # then the two half-K kpos-4 MMs run concurrently in
                            # disjoint (row_grp, col_grp) array tiles.
                            ps0 = ppool.tile([128, STRIP_ROWS, HP], f32, tag="ps")
                            ps1 = ppool.tile([128, STRIP_ROWS, HP], f32, tag="ps")
                            pss = [ps0, ps1]
                            pf0 = ps0.rearrange("p r w -> p (r w)")
                            pf1 = ps1.rearrange("p r w -> p (r w)")
                            for kidx, k in enumerate((0, 1, 2, 3)):
                                nc.tensor.matmul(
                                    pf0[:, :nmm], wt_r[:, k, :], span(s, k)[0],
                                    start=(kidx == 0), stop=False)
                            for kidx, k in enumerate((5, 6, 7, 8)):
                                nc.tensor.matmul(
                                    pf1[:, :nmm], wt_r[:, k, :], span(s, k)[0],
                                    start=(kidx == 0), stop=False)
                            _, st4 = span(s, 4)
                            nc.tensor.matmul(
                                pf0[:, :nmm], wt_r[0:64, 4, :],
                                xp[0:64, st4:st4 + nmm],
                                start=False, stop=True,
                                tile_position=(0, 0), skip_group_check=True)
                            nc.tensor.matmul(
                                pf1[:, :nmm], wt_r[64:128, 4, :],
                                xp[64:128, st4:st4 + nmm],
                                start=False, stop=True,
                                tile_position=(64, 0), skip_group_check=True)
                        for chunk in range(2) if mode != 'dma_only' else ():
                            dst = out_sb[:, chunk, s * N_OUT:(s + 1) * N_OUT] \
                                .rearrange("p (r w) -> p r w", w=W)
                            if drain_split and chunk == 1:
                                nc.scalar.activation(
                                    dst, pss[chunk][:, :, 0:W],
                                    mybir.ActivationFunctionType.Identity,
                                    bias=bias_sb[:, chunk:chunk + 1], scale=1.0)
                            else:
                                nc.vector.tensor_scalar_add(
                                    dst, pss[chunk][:, :, 0:W],
                                    bias_sb[:, chunk:chunk + 1],
                                )
                        if s in OUT_SPLITS:
                            lo, hi = OUT_SPLITS[s]
                            if mode in ('no_out', 'pe_only'):
                                if s == 6:
                                    nc.scalar.dma_start(out=ys[b, 0, :, :16],
                                                        in_=out_sb[:, 0, :16])
                            else:
                                eng = nc.gpsimd if (out_pool or
                                                    (opt and (b + s) % 2)) \
                                    else nc.scalar
                                if out_split2:
                                    # every piece: halves on two queues; the
                                    # final piece optionally rides the two
                                    # low-latency HWDGE rings (sync is idle
                                    # at image end) to shrink the tail
                                    mid = (lo + hi) // 2
                                    e1 = nc.sync if (tail_sync and
                                                     s == N_STRIPS - 1) \
                                        else nc.gpsimd
                                    for e, l2, h2 in ((e1, lo, mid),
                                                      (nc.scalar, mid, hi)):
                                        e.dma_start(
                                            out=ys[b, :, :, l2:h2]
                                            .rearrange("c2 p hw -> p c2 hw"),
                                            in_=out_sb[:, :, l2:h2])
                                elif opt and s == N_STRIPS - 1:
                                    # final piece: halves on two queues to
                                    # shrink the kernel tail
                                    mid = (lo + hi) // 2
                                    for e, l2, h2 in ((nc.gpsimd, lo, mid),
                                                      (nc.scalar, mid, hi)):
                                        e.dma_start(
                                            out=ys[b, :, :, l2:h2]
                                            .rearrange("c2 p hw -> p c2 hw"),
                                            in_=out_sb[:, :, l2:h2])
                                else:
                                    eng.dma_start(
                                        out=ys[b, :, :, lo:hi]
                                        .rearrange("c2 p hw -> p c2 hw"),
                                        in_=out_sb[:, :, lo:hi])

            if repeat == 1:
                body()
            else:
                n_loop = repeat // unroll
                if n_loop > 0:
                    with tc.For_i(0, n_loop, 1,
                                  hint_engines=(mybir.EngineType.PE,)):
                        for _ in range(unroll):
                            body()
                for _ in range(repeat - n_loop * unroll):
                    body()
    nc.finalize()
    return nc


def _get_runner(repeat: int = 1):
    global _RUNNER
    if _RUNNER is None or _RUNNER[0] != repeat:
        from bass_exec_inline import BassRunner
        nc = _build_nc(repeat)
        _RUNNER = (repeat, BassRunner(nc, n_cores=N_CORES))
    return _RUNNER[1]


def _prep_params(twiddle: np.ndarray, bias: np.ndarray, k4merge: bool = True,
                 in_dt: str = 'bf16', scheme: str = 'grp32'):
    W_dense = _compose_w(np.asarray(twiddle))
    if scheme == 'grp32':
        # group g (outs 32g..32g+31) = sum_j W[32g:+32, 128(g+j):+128] @ x_kpos
        # block-diagonality makes rows outside [144g, 144g+144) exactly zero
        wts = np.zeros((16, C_IN, 32), np.float32)
        for g in range(8):
            for j in range(2):
                c0 = 128 * (g + j)
                wts[2 * g + j] = W_dense[32 * g:32 * g + 32,
                                         c0:c0 + 128].T.astype(np.float32)
        biasT = np.asarray(bias, np.float32).reshape(2, 128).T.copy()
        if in_dt == 'bf16':
            import ml_dtypes
            wts = wts.astype(ml_dtypes.bfloat16)
        return wts, biasT
    wts = np.zeros((10, C_IN, 128), np.float32)
    if k4merge:
        # slot k (k != 4): full W slice for kpos k into its chunk
        for k in range(9):
            if k == 4:
                continue
            chunk = 0 if k < 4 else 1
            blk = W_dense[chunk * 128:(chunk + 1) * 128, 128 * k:128 * (k + 1)]
            wts[k] = blk.T.astype(np.float32)
        # slot 4 packed for the row-tiled pair: ch 0..63 carry chunk0's kpos-4
        # weights (full co 0..127), ch 64..127 carry chunk1's (co 128..255)
        wts[4][0:64, :] = W_dense[0:128, 512:576].T.astype(np.float32)
        wts[4][64:128, :] = W_dense[128:256, 576:640].T.astype(np.float32)
    else:
        for i, (chunk, k) in enumerate(CHUNK_KPOS):
            blk = W_dense[chunk * 128:(chunk + 1) * 128, 128 * k:128 * (k + 1)]
            wts[i] = blk.T.astype(np.float32)
    biasT = np.asarray(bias, np.float32).reshape(2, 128).T.copy()
    if in_dt == 'bf16':
        import ml_dtypes
        wts = wts.astype(ml_dtypes.bfloat16)
    return wts, biasT


def _prep_x(x: np.ndarray, in_dt: str = 'bf16') -> np.ndarray:
    """(32,128,56,56) -> column-padded (8, 4, 128, 56*58); the top/bottom
    pad rows live as persistent zeros in SBUF (never transferred)."""
    x = np.asarray(x, np.float32).reshape(B, C_IN, H, W)
    dt = np.float32
    if in_dt == 'bf16':
        import ml_dtypes
        dt = ml_dtypes.bfloat16
    xp = np.zeros((B, C_IN, H, HP), dt)
    xp[:, :, :, 1:1 + W] = x.astype(dt)
    return xp.reshape(N_CORES, B_LOC, C_IN, H * HP)


def kernel(x: np.ndarray, twiddle: np.ndarray, bias: np.ndarray) -> np.ndarray:
    wts, biasT = _prep_params(twiddle, bias)
    runner = _get_runner(1)
    xsh = _prep_x(x)
    in_maps = [{"xs": xsh[c], "wts": wts, "biasT": biasT} for c in range(N_CORES)]
    res = runner(runner.pack(in_maps))
    out = np.stack([res[c]["ys"] for c in range(N_CORES)])  # (8,4,2,128,3136)
    return out.reshape(B, C_OUT, H, W).astype(np.float32)


# ---- inline copy of the reusable jitted runner (kernel.py self-contained) --
import sys as _sys
import types as _types

_BASS_EXEC_SRC = '''
import numpy as np
import jax
from jax.sharding import Mesh, PartitionSpec
from jax.experimental.shard_map import shard_map

import concourse.mybir as mybir
from concourse.bass2jax import _bass_exec_p, partition_id_tensor, install_neuronx_cc_hook


class BassRunner:
    def __init__(self, nc, n_cores=8):
        install_neuronx_cc_hook()
        assert nc.is_finalized()
        self.nc = nc
        self.n_cores = n_cores
        partition_name = nc.partition_id_tensor.name if nc.partition_id_tensor else None

        in_names, out_names, out_avals, zero_outs = [], [], [], []
        for alloc in nc.m.functions[0].allocations:
            if not isinstance(alloc, mybir.MemoryLocationSet):
                continue
            name = alloc.memorylocations[0].name
            if alloc.kind == "ExternalInput":
                if name != partition_name:
                    in_names.append(name)
            elif alloc.kind == "ExternalOutput":
                out_names.append(name)
                shape = tuple(alloc.tensor_shape)
                dtype = mybir.dt.np(alloc.dtype)
                out_avals.append(jax.core.ShapedArray(shape, dtype))
                zero_outs.append(np.zeros(shape, dtype))
        self.n_params = len(in_names)
        n_outs = len(out_avals)
        self.in_names = list(in_names)
        self.out_names = out_names
        self.out_avals = out_avals
        self.zero_outs = zero_outs
        all_in_names = in_names + out_names
        if partition_name is not None:
            all_in_names.append(partition_name)

        donate = tuple(range(self.n_params, self.n_params + n_outs))

        def _body(*args):
            operands = list(args)
            if partition_name is not None:
                operands.append(partition_id_tensor())
            outs = _bass_exec_p.bind(
                *operands,
                out_avals=tuple(out_avals),
                in_names=tuple(all_in_names),
                out_names=tuple(out_names),
                lowering_input_output_aliases=(),
                sim_require_finite=True,
                sim_require_nnan=True,
                nc=nc,
            )
            return tuple(outs)

        devices = jax.devices()[:n_cores]
        mesh = Mesh(np.asarray(devices), ("core",))
        self._mesh = mesh
        self._zeros_fn = None
        in_specs = (PartitionSpec("core"),) * (self.n_params + n_outs)
        out_specs = (PartitionSpec("core"),) * len(out_names)
        self._fn = jax.jit(
            shard_map(_body, mesh=mesh, in_specs=in_specs, out_specs=out_specs,
                      check_rep=False),
            donate_argnums=donate, keep_unused=True,
        )


    def pack_device(self, in_maps):
        """device_put the packed inputs once; reuse across calls."""
        import jax.numpy as jnp
        from jax.sharding import NamedSharding
        concat = self.pack(in_maps)
        sh = NamedSharding(self._mesh, PartitionSpec("core"))
        return [jax.device_put(a, sh) for a in concat]

    def zeros_device(self):
        if self._zeros_fn is None:
            import jax.numpy as jnp
            from jax.sharding import NamedSharding
            sh = NamedSharding(self._mesh, PartitionSpec("core"))
            shapes = [(self.n_cores * z.shape[0], *z.shape[1:]) for z in self.zero_outs]
            dts = [z.dtype for z in self.zero_outs]

            def _mk():
                return tuple(jnp.zeros(s, d) for s, d in zip(shapes, dts))
            self._zeros_fn = jax.jit(_mk, out_shardings=tuple([sh] * len(shapes)))
        return self._zeros_fn()

    def call_device(self, concat_in_dev):
        """Device-resident call: returns raw jax output arrays."""
        zeros = self.zeros_device()
        return self._fn(*concat_in_dev, *zeros)

    def pack(self, in_maps):
        per_core = [[np.asarray(m[name]) for name in self.in_names] for m in in_maps]
        return [
            np.concatenate([per_core[c][i] for c in range(self.n_cores)], axis=0)
            for i in range(self.n_params)
        ]

    def __call__(self, concat_in, raw=False):
        concat_zeros = [
            np.zeros((self.n_cores * z.shape[0], *z.shape[1:]), z.dtype)
            for z in self.zero_outs
        ]
        out_arrs = self._fn(*concat_in, *concat_zeros)
        if raw:
            return out_arrs
        return [
            {
                name: np.asarray(out_arrs[i]).reshape(
                    self.n_cores, *self.out_avals[i].shape)[c]
                for i, name in enumerate(self.out_names)
            }
            for c in range(self.n_cores)
        ]
'''

_mod = _types.ModuleType("bass_exec_inline")
exec(compile(_BASS_EXEC_SRC, "bass_exec_inline", "exec"), _mod.__dict__)
_sys.modules["bass_exec_inline"] = _mod



# revision 19
# speedup vs baseline: 1.0060x; 1.0060x over previous
"""DeBut 2D conv (32,128,56,56) -> (32,256,56,56) on 8 axon TRN2 NeuronCores.

The butterfly product W3@W2@W1 composes to a block-diagonal (256,1152) matrix
with 32 blocks of (8 out x 36 in). Finer than the 2-chunk split: output
GROUP g of 32 channels (32g..32g+31, g=0..7) depends only on input features
144g..144g+143, which live inside kernel positions g and g+1. So group g is
exactly TWO kpos-pure K=128 matmuls with weights W[32g:+32, 128(g+j):+128].T
(j=0,1) - the zero rows come for free from block-diagonality.

The PE array is addressed in 128x32 column-tiled mode: 4 col quadrants run
4 concurrent M=32 matmuls, each streaming its own moving span. Per strip
that is 16 matmuls in 4 concurrent rounds = 4x462 PE cycles instead of the
9x462 of the 2-chunk scheme (2.25x less PE stream time), pushing the kernel
from PE-bound to the DMA roofline.

Per-core layout (batch sharded 4 images/core):
  x arrives host-padded to 56x58 (left/right zero cols only), converted to
  bf16 (rel-err budget is 2e-2; full-bf16 pipeline lands ~4e-3), and is
  DMA'd in 4 strip-aligned cuts per image into rows 1..56 of one of 4
  resident SBUF slots; top/bottom pad rows are persistent SBUF zeros.
  bf16 halves both DMA directions; the PE streams 1 col/cycle either way.
  7 strips of 8 output rows; moving operand = contiguous 462-col span of the
  padded image; psum (128, 8, 58) f32; drain cols 0:56 + bias via DVE
  tensor_scalar_add (chunk0) / ACT activation (chunk1), casting to bf16.
  Output pieces are split across BOTH the SWDGE (gpsimd) and ACT HWDGE
  queues; inputs ride the SP HWDGE queue.
  The repeat loop (timing harness) is unrolled 8x inside tc.For_i: each
  For_i iteration ends in a full 5-engine + DMA barrier (~5-8us), so deep
  unroll amortizes it; repeat counts stay exact via trailing bodies.
"""
import numpy as np

# ---- problem constants (hardcoded; kernel.py must be self-contained) ----
B, C_IN, H, W = 32, 128, 56, 56
C_OUT = 256
KS = 3
N_CORES = 8
B_LOC = B // N_CORES          # 4 images per core
HP = H + 2                    # 58
PADDED = HP * HP + 8          # 3372, slack for junk-column overreads
STRIP_ROWS = 8
N_STRIPS = H // STRIP_ROWS    # 7
N_MM = STRIP_ROWS * HP        # 464 moving columns per matmul
N_OUT = STRIP_ROWS * W        # 448 valid columns per strip
R_SHAPES = [(768, 1152, 2, 3, 1), (512, 768, 2, 3, 2), (256, 512, 2, 4, 4)]
# (chunk, kpos) pairs: chunk0 -> kpos 0..4, chunk1 -> kpos 4..8
CHUNK_KPOS = [(0, k) for k in range(5)] + [(1, k) for k in range(4, 9)]

_RUNNER = None


def _compose_w(twiddle: np.ndarray) -> np.ndarray:
    """Compose butterfly factors into the dense (256, 1152) matrix (float64)."""
    W_full = None
    temp = 0
    for (osz, isz, row, col, diag) in R_SHAPES:
        npar = col * osz
        nb = isz // (col * diag)
        t = twiddle[temp:temp + npar].astype(np.float64)
        t = t.reshape(nb, diag, row, col).transpose(0, 2, 3, 1)  # (n, r, c, d)
        temp += npar
        Ws = np.zeros((osz, isz), np.float64)
        # out index n*row*diag + r*diag + d ; in index n*col*diag + c*diag + d
        for d in range(diag):
            for r in range(row):
                for c in range(col):
                    out_idx = np.arange(nb) * row * diag + r * diag + d
                    in_idx = np.arange(nb) * col * diag + c * diag + d
                    Ws[out_idx, in_idx] = t[:, r, c, d]
        W_full = Ws if W_full is None else Ws @ W_full
    return W_full  # (256, 1152)


def _build_nc(repeat: int = 1, trace_sim: bool = False, mode: str = 'full',
              outp_bufs: int = 3, psum_bufs: int = 6, k4merge: bool = True,
              n_xpad: int = 4, drain_split: bool = True, opt: bool = True,
              out_pool: bool = True, early_out: bool = False,
              out_dt: str = 'bf16', out_split2: bool = True,
              in_dt: str = 'bf16', in_cuts: int = 1, unroll: int = 8,
              cuts2: bool = True, tail_sync: bool = False,
              scheme: str = 'grp32', dma4q: bool = False):
    import concourse.bass as bass  # noqa: F401
    from concourse import bacc
    import concourse.mybir as mybir
    from concourse.tile import TileContext

    f32 = mybir.dt.float32
    f32r = mybir.dt.float32r
    bf16 = mybir.dt.bfloat16

    nc = bacc.Bacc("TRN2", target_bir_lowering=False, debug=False,
                   num_devices=N_CORES)
    # xs/wts are declared float32r: same 4-byte layout (numpy float32 binds),
    # lets plain HWDGE DMAs feed the f32r matmuls with no cast pass.
    # xs arrives column-padded to 56x58 so the in-DMA is one contiguous
    # ~13KB run per partition; top/bottom pad rows stay resident zeros.
    idt = bf16 if in_dt == 'bf16' else f32r
    xs = nc.declare_dram_parameter("xs", [B_LOC, C_IN, H * HP], idt,
                                   isOutput=False)
    # grp32: 16 weight mats [128 in-ch, 32 outs], slot 2g+j = group g piece j
    NW, WM = (16, 32) if scheme == 'grp32' else (10, 128)
    wts = nc.declare_dram_parameter("wts", [NW, C_IN, WM], idt, isOutput=False)
    biasT = nc.declare_dram_parameter("biasT", [128, 2], f32, isOutput=False)
    # output stored bf16: halves out-DMA traffic (rel-err budget is 2e-2,
    # bf16 rounding adds ~2e-3); host upcasts to f32
    odt = bf16 if out_dt == 'bf16' else f32
    ys = nc.declare_dram_parameter("ys", [B_LOC, 2, 128, H * W], odt,
                                   isOutput=True)

    K4MERGE = k4merge
    with TileContext(nc, trace_sim=trace_sim) as tc:
        with tc.tile_pool(name="sbuf", bufs=1) as cpool, \
             tc.tile_pool(name="outp", bufs=outp_bufs) as opool, \
             tc.tile_pool(name="psum", bufs=psum_bufs, space="PSUM") as ppool:
            # persistent padded-image slots; 8-col slack zeroed once below
            N_XPAD = n_xpad
            xpads = [cpool.tile([C_IN, PADDED], idt, tag=f"xpad{i}",
                                name=f"xpad{i}")
                     for i in range(N_XPAD)]
            zrow = cpool.tile([C_IN, HP + 8], f32 if in_dt != 'bf16' else bf16,
                              tag="zrow")
            nc.vector.memset(zrow[:], 0.0)
            for xp in xpads:
                nc.vector.tensor_copy(xp[:, :HP], zrow[:, :HP])   # top pad row
                # bottom pad row + slack (never overwritten by image DMAs)
                nc.vector.tensor_copy(xp[:, (HP - 1) * HP:], zrow[:])
            # first image on the SP HWDGE queue, weights+bias on the ACT HWDGE
            # queue - they land in parallel
            # strip s reads image rows 8s-1..8s+9; boundaries (10,26,42)
            # stagger the per-strip waits as c0,c1,c1,c2,c2,c3,c3
            CUTS = ((0, 10 * HP), (10 * HP, 26 * HP), (26 * HP, 42 * HP),
                    (42 * HP, H * HP)) if cuts2 else \
                   ((0, 9 * HP), (9 * HP, 24 * HP), (24 * HP, 40 * HP),
                    (40 * HP, H * HP))
            # dma4q: each HWDGE queue sustains only ~125 GB/s, so balance the
            # 9.85MB/body across four queues: in cuts 0-2 on sync, cut 3 on
            # vector; out pieces spread over gpsimd/scalar/vector below.
            def in_eng(i):
                return nc.vector if (dma4q and i == 3) else nc.sync

            if mode not in ('no_in', 'pe_only', 'dma_out'):
                cuts = CUTS if opt else ((0, 34 * HP), (34 * HP, H * HP))
                for i, (lo, hi) in enumerate(cuts):
                    in_eng(i).dma_start(out=xpads[0][:, HP + lo:HP + hi],
                                        in_=xs[0, :, lo:hi])
            wt_r = cpool.tile([C_IN, NW, WM], idt, tag="wtr")
            # chunk0's slots land first so the first matmul group can start
            # before the full weight transfer completes
            nc.scalar.dma_start(out=wt_r[:, 0:NW // 2, :],
                                in_=wts.ap()[0:NW // 2]
                                .rearrange("i c m -> c i m"))
            nc.scalar.dma_start(out=wt_r[:, NW // 2:NW, :],
                                in_=wts.ap()[NW // 2:NW]
                                .rearrange("i c m -> c i m"))
            bias_sb = cpool.tile([128, 2], f32, tag="bias")
            nc.scalar.dma_start(out=bias_sb[:], in_=biasT.ap())
            scratch = cpool.tile([C_IN, H * HP], idt, tag="scr",
                                 name="scr") \
                if mode == 'scratch_in' else None

            # out-DMA split points; spreading pieces across the image keeps
            # the out queues busy from strip 1 on and shrinks the tail
            if dma4q:
                early_out = 2
            if early_out == 3:
                OUT_SPLITS = {s: (s * N_OUT, (s + 1) * N_OUT)
                              for s in range(N_STRIPS)}
            elif early_out == 2:
                OUT_SPLITS = {1: (0, 2 * N_OUT), 3: (2 * N_OUT, 4 * N_OUT),
                              5: (4 * N_OUT, 6 * N_OUT),
                              6: (6 * N_OUT, 7 * N_OUT)}
            elif early_out:
                OUT_SPLITS = {2: (0, 3 * N_OUT), 4: (3 * N_OUT, 5 * N_OUT),
                              5: (5 * N_OUT, 6 * N_OUT),
                              6: (6 * N_OUT, 7 * N_OUT)}
            else:
                OUT_SPLITS = {3: (0, 4 * N_OUT), 5: (4 * N_OUT, 6 * N_OUT),
                              6: (6 * N_OUT, 7 * N_OUT)}

            def body():
                for b in range(B_LOC):
                    if mode == 'scratch_in' and (b > 0 or repeat > 1):
                        # same DMA traffic, but no PE dependency: writes go to
                        # a scratch tile nobody reads (concurrency probe)
                        nc.sync.dma_start(out=scratch[:], in_=xs[b])
                    elif (b > 0 or repeat > 1) and mode not in ('no_in',
                                                                'pe_only',
                                                                'dma_out'):
                        # in_cuts: 1 = cut every image, 2 = cut only the
                        # first (post-barrier) image — later images hide
                        # under compute, and fewer DMAs means less
                        # completion-semaphore lane aliasing
                        if in_cuts == 1 or (in_cuts == 2 and b == 0):
                            # strip-aligned pieces: first strip's matmuls wait
                            # only the first cut, not the whole image
                            for i, (lo, hi) in enumerate(CUTS):
                                in_eng(i).dma_start(
                                    out=xpads[b % N_XPAD][:, HP + lo:HP + hi],
                                    in_=xs[b, :, lo:hi])
                        else:
                            nc.sync.dma_start(
                                out=xpads[b % N_XPAD][:, HP:HP + H * HP],
                                in_=xs[b])

                    xp = xpads[b % N_XPAD]

                    nmm = N_MM - 2 if opt else N_MM

                    def span(s, k):
                        di, dj = k // 3, k % 3
                        st = (STRIP_ROWS * s + di) * HP + dj
                        return xp[:, st:st + nmm], st

                    out_sb = opool.tile([128, 2, H * W], odt, tag="outsb")
                    if mode in ('dma_only', 'dma_in', 'dma_out'):
                        nc.vector.memset(out_sb[:, :, 0:4], 0.0)
                    for s in range(N_STRIPS):
                        if mode in ('dma_only', 'dma_in', 'dma_out'):
                            pass
                        elif scheme == 'grp32':
                            # 16 col-tiled matmuls (128x32 mode): group g ->
                            # psum quadrant g%4 of chunk g//4; piece j streams
                            # the kpos-(g+j) span. Issue in 4-lane rounds so
                            # the 4 col quadrants run concurrently.
                            ps0 = ppool.tile([128, STRIP_ROWS, HP], f32,
                                             tag="ps")
                            ps1 = ppool.tile([128, STRIP_ROWS, HP], f32,
                                             tag="ps")
                            pss = [ps0, ps1]
                            pfs = [p.rearrange("p r w -> p (r w)") for p in pss]
                            for chunk in range(2):
                                for j in range(2):
                                    for q in range(4):
                                        g = chunk * 4 + q
                                        nc.tensor.matmul(
                                            pfs[chunk][32 * q:32 * q + 32,
                                                       :nmm],
                                            wt_r[:, 2 * g + j, :],
                                            span(s, g + j)[0],
                                            start=(j == 0), stop=(j == 1),
                                            tile_position=(0, 32 * q),
                                            skip_group_check=True)
                        elif not K4MERGE:
                            pss = []
                            for chunk in range(2):
                                ps = ppool.tile([128, STRIP_ROWS, HP], f32,
                                                tag="ps")
                                pss.append(ps)
                                psflat = ps.rearrange("p r w -> p (r w)")
                                for kidx in range(5):
                                    _, k = CHUNK_KPOS[chunk * 5 + kidx]
                                    nc.tensor.matmul(
                                        psflat[:, :nmm],
                                        wt_r[:, chunk * 5 + kidx, :],
                                        span(s, k)[0],
                                        start=(kidx == 0), stop=(kidx == 4))
                        else:
                            # chunk0 <- k0..3, chunk1 <- k5..8 (full-array MMs),
                            # then the two half-K kpos-4 MMs run concurrently in
                            # disjoint (row_grp, col_grp) array tiles.
                            ps0 = ppool.tile([128, STRIP_ROWS, HP], f32, tag="ps")
                            ps1 = ppool.tile([128, STRIP_ROWS, HP], f32, tag="ps")
                            pss = [ps0, ps1]
                            pf0 = ps0.rearrange("p r w -> p (r w)")
                            pf1 = ps1.rearrange("p r w -> p (r w)")
                            for kidx, k in enumerate((0, 1, 2, 3)):
                                nc.tensor.matmul(
                                    pf0[:, :nmm], wt_r[:, k, :], span(s, k)[0],
                                    start=(kidx == 0), stop=False)
                            for kidx, k in enumerate((5, 6, 7, 8)):
                                nc.tensor.matmul(
                                    pf1[:, :nmm], wt_r[:, k, :], span(s, k)[0],
                                    start=(kidx == 0), stop=False)
                            _, st4 = span(s, 4)
                            nc.tensor.matmul(
                                pf0[:, :nmm], wt_r[0:64, 4, :],
                                xp[0:64, st4:st4 + nmm],
                                start=False, stop=True,
                                tile_position=(0, 0), skip_group_check=True)
                            nc.tensor.matmul(
                                pf1[:, :nmm], wt_r[64:128, 4, :],
                                xp[64:128, st4:st4 + nmm],
                                start=False, stop=True,
                                tile_position=(64, 0), skip_group_check=True)
                        for chunk in range(2) if mode not in ('dma_only', 'dma_in', 'dma_out') else ():
                            dst = out_sb[:, chunk, s * N_OUT:(s + 1) * N_OUT] \
                                .rearrange("p (r w) -> p r w", w=W)
                            if drain_split and chunk == 1:
                                nc.scalar.activation(
                                    dst, pss[chunk][:, :, 0:W],
                                    mybir.ActivationFunctionType.Identity,
                                    bias=bias_sb[:, chunk:chunk + 1], scale=1.0)
                            else:
                                nc.vector.tensor_scalar_add(
                                    dst, pss[chunk][:, :, 0:W],
                                    bias_sb[:, chunk:chunk + 1],
                                )
                        if s in OUT_SPLITS:
                            lo, hi = OUT_SPLITS[s]
                            if mode in ('no_out', 'pe_only', 'dma_in'):
                                if s == 6:
                                    nc.scalar.dma_start(out=ys[b, 0, :, :16],
                                                        in_=out_sb[:, 0, :16])
                            elif dma4q:
                                # per image: s1 -> gpsimd, s3 -> scalar,
                                # s5 -> half gpsimd + half scalar, s6 -> vector
                                # (vector also carries in-cut 3) ~0.6MB/queue
                                if s == 5:
                                    mid = (lo + hi) // 2
                                    plan = ((nc.gpsimd, lo, mid),
                                            (nc.scalar, mid, hi))
                                else:
                                    e = {1: nc.gpsimd, 3: nc.scalar,
                                         6: nc.vector}[s]
                                    plan = ((e, lo, hi),)
                                for e, l2, h2 in plan:
                                    e.dma_start(
                                        out=ys[b, :, :, l2:h2]
                                        .rearrange("c2 p hw -> p c2 hw"),
                                        in_=out_sb[:, :, l2:h2])
                            else:
                                eng = nc.gpsimd if (out_pool or
                                                    (opt and (b + s) % 2)) \
                                    else nc.scalar
                                if out_split2:
                                    # every piece: halves on two queues; the
                                    # final piece optionally rides the two
                                    # low-latency HWDGE rings (sync is idle
                                    # at image end) to shrink the tail
                                    mid = (lo + hi) // 2
                                    e1 = nc.sync if (tail_sync and
                                                     s == N_STRIPS - 1) \
                                        else nc.gpsimd
                                    for e, l2, h2 in ((e1, lo, mid),
                                                      (nc.scalar, mid, hi)):
                                        e.dma_start(
                                            out=ys[b, :, :, l2:h2]
                                            .rearrange("c2 p hw -> p c2 hw"),
                                            in_=out_sb[:, :, l2:h2])
                                elif opt and s == N_STRIPS - 1:
                                    # final piece: halves on two queues to
                                    # shrink the kernel tail
                                    mid = (lo + hi) // 2
                                    for e, l2, h2 in ((nc.gpsimd, lo, mid),
                                                      (nc.scalar, mid, hi)):
                                        e.dma_start(
                                            out=ys[b, :, :, l2:h2]
                                            .rearrange("c2 p hw -> p c2 hw"),
                                            in_=out_sb[:, :, l2:h2])
                                else:
                                    eng.dma_start(
                                        out=ys[b, :, :, lo:hi]
                                        .rearrange("c2 p hw -> p c2 hw"),
                                        in_=out_sb[:, :, lo:hi])

            if repeat == 1:
                body()
            else:
                n_loop = repeat // unroll
                if n_loop > 0:
                    with tc.For_i(0, n_loop, 1,
                                  hint_engines=(mybir.EngineType.PE,)):
                        for _ in range(unroll):
                            body()
                for _ in range(repeat - n_loop * unroll):
                    body()
    nc.finalize()
    return nc


def _get_runner(repeat: int = 1):
    global _RUNNER
    if _RUNNER is None or _RUNNER[0] != repeat:
        from bass_exec_inline import BassRunner
        nc = _build_nc(repeat)
        _RUNNER = (repeat, BassRunner(nc, n_cores=N_CORES))
    return _RUNNER[1]


def _prep_params(twiddle: np.ndarray, bias: np.ndarray, k4merge: bool = True,
                 in_dt: str = 'bf16', scheme: str = 'grp32'):
    W_dense = _compose_w(np.asarray(twiddle))
    if scheme == 'grp32':
        # group g (outs 32g..32g+31) = sum_j W[32g:+32, 128(g+j):+128] @ x_kpos
        # block-diagonality makes rows outside [144g, 144g+144) exactly zero
        wts = np.zeros((16, C_IN, 32), np.float32)
        for g in range(8):
            for j in range(2):
                c0 = 128 * (g + j)
                wts[2 * g + j] = W_dense[32 * g:32 * g + 32,
                                         c0:c0 + 128].T.astype(np.float32)
        biasT = np.asarray(bias, np.float32).reshape(2, 128).T.copy()
        if in_dt == 'bf16':
            import ml_dtypes
            wts = wts.astype(ml_dtypes.bfloat16)
        return wts, biasT
    wts = np.zeros((10, C_IN, 128), np.float32)
    if k4merge:
        # slot k (k != 4): full W slice for kpos k into its chunk
        for k in range(9):
            if k == 4:
                continue
            chunk = 0 if k < 4 else 1
            blk = W_dense[chunk * 128:(chunk + 1) * 128, 128 * k:128 * (k + 1)]
            wts[k] = blk.T.astype(np.float32)
        # slot 4 packed for the row-tiled pair: ch 0..63 carry chunk0's kpos-4
        # weights (full co 0..127), ch 64..127 carry chunk1's (co 128..255)
        wts[4][0:64, :] = W_dense[0:128, 512:576].T.astype(np.float32)
        wts[4][64:128, :] = W_dense[128:256, 576:640].T.astype(np.float32)
    else:
        for i, (chunk, k) in enumerate(CHUNK_KPOS):
            blk = W_dense[chunk * 128:(chunk + 1) * 128, 128 * k:128 * (k + 1)]
            wts[i] = blk.T.astype(np.float32)
    biasT = np.asarray(bias, np.float32).reshape(2, 128).T.copy()
    if in_dt == 'bf16':
        import ml_dtypes
        wts = wts.astype(ml_dtypes.bfloat16)
    return wts, biasT


def _prep_x(x: np.ndarray, in_dt: str = 'bf16') -> np.ndarray:
    """(32,128,56,56) -> column-padded (8, 4, 128, 56*58); the top/bottom
    pad rows live as persistent zeros in SBUF (never transferred)."""
    x = np.asarray(x, np.float32).reshape(B, C_IN, H, W)
    dt = np.float32
    if in_dt == 'bf16':
        import ml_dtypes
        dt = ml_dtypes.bfloat16
    xp = np.zeros((B, C_IN, H, HP), dt)
    xp[:, :, :, 1:1 + W] = x.astype(dt)
    return xp.reshape(N_CORES, B_LOC, C_IN, H * HP)


def kernel(x: np.ndarray, twiddle: np.ndarray, bias: np.ndarray) -> np.ndarray:
    wts, biasT = _prep_params(twiddle, bias)
    runner = _get_runner(1)
    xsh = _prep_x(x)
    in_maps = [{"xs": xsh[c], "wts": wts, "biasT": biasT} for c in range(N_CORES)]
    res = runner(runner.pack(in_maps))
    out = np.stack([res[c]["ys"] for c in range(N_CORES)])  # (8,4,2,128,3136)
    return out.reshape(B, C_OUT, H, W).astype(np.float32)


# ---- inline copy of the reusable jitted runner (kernel.py self-contained) --
import sys as _sys
import types as _types

_BASS_EXEC_SRC = '''
import numpy as np
import jax
from jax.sharding import Mesh, PartitionSpec
from jax.experimental.shard_map import shard_map

import concourse.mybir as mybir
from concourse.bass2jax import _bass_exec_p, partition_id_tensor, install_neuronx_cc_hook


class BassRunner:
    def __init__(self, nc, n_cores=8):
        install_neuronx_cc_hook()
        assert nc.is_finalized()
        self.nc = nc
        self.n_cores = n_cores
        partition_name = nc.partition_id_tensor.name if nc.partition_id_tensor else None

        in_names, out_names, out_avals, zero_outs = [], [], [], []
        for alloc in nc.m.functions[0].allocations:
            if not isinstance(alloc, mybir.MemoryLocationSet):
                continue
            name = alloc.memorylocations[0].name
            if alloc.kind == "ExternalInput":
                if name != partition_name:
                    in_names.append(name)
            elif alloc.kind == "ExternalOutput":
                out_names.append(name)
                shape = tuple(alloc.tensor_shape)
                dtype = mybir.dt.np(alloc.dtype)
                out_avals.append(jax.core.ShapedArray(shape, dtype))
                zero_outs.append(np.zeros(shape, dtype))
        self.n_params = len(in_names)
        n_outs = len(out_avals)
        self.in_names = list(in_names)
        self.out_names = out_names
        self.out_avals = out_avals
        self.zero_outs = zero_outs
        all_in_names = in_names + out_names
        if partition_name is not None:
            all_in_names.append(partition_name)

        donate = tuple(range(self.n_params, self.n_params + n_outs))

        def _body(*args):
            operands = list(args)
            if partition_name is not None:
                operands.append(partition_id_tensor())
            outs = _bass_exec_p.bind(
                *operands,
                out_avals=tuple(out_avals),
                in_names=tuple(all_in_names),
                out_names=tuple(out_names),
                lowering_input_output_aliases=(),
                sim_require_finite=True,
                sim_require_nnan=True,
                nc=nc,
            )
            return tuple(outs)

        devices = jax.devices()[:n_cores]
        mesh = Mesh(np.asarray(devices), ("core",))
        self._mesh = mesh
        self._zeros_fn = None
        in_specs = (PartitionSpec("core"),) * (self.n_params + n_outs)
        out_specs = (PartitionSpec("core"),) * len(out_names)
        self._fn = jax.jit(
            shard_map(_body, mesh=mesh, in_specs=in_specs, out_specs=out_specs,
                      check_rep=False),
            donate_argnums=donate, keep_unused=True,
        )


    def pack_device(self, in_maps):
        """device_put the packed inputs once; reuse across calls."""
        import jax.numpy as jnp
        from jax.sharding import NamedSharding
        concat = self.pack(in_maps)
        sh = NamedSharding(self._mesh, PartitionSpec("core"))
        return [jax.device_put(a, sh) for a in concat]

    def zeros_device(self):
        if self._zeros_fn is None:
            import jax.numpy as jnp
            from jax.sharding import NamedSharding
            sh = NamedSharding(self._mesh, PartitionSpec("core"))
            shapes = [(self.n_cores * z.shape[0], *z.shape[1:]) for z in self.zero_outs]
            dts = [z.dtype for z in self.zero_outs]

            def _mk():
                return tuple(jnp.zeros(s, d) for s, d in zip(shapes, dts))
            self._zeros_fn = jax.jit(_mk, out_shardings=tuple([sh] * len(shapes)))
        return self._zeros_fn()

    def call_device(self, concat_in_dev):
        """Device-resident call: returns raw jax output arrays."""
        zeros = self.zeros_device()
        return self._fn(*concat_in_dev, *zeros)

    def pack(self, in_maps):
        per_core = [[np.asarray(m[name]) for name in self.in_names] for m in in_maps]
        return [
            np.concatenate([per_core[c][i] for c in range(self.n_cores)], axis=0)
            for i in range(self.n_params)
        ]

    def __call__(self, concat_in, raw=False):
        concat_zeros = [
            np.zeros((self.n_cores * z.shape[0], *z.shape[1:]), z.dtype)
            for z in self.zero_outs
        ]
        out_arrs = self._fn(*concat_in, *concat_zeros)
        if raw:
            return out_arrs
        return [
            {
                name: np.asarray(out_arrs[i]).reshape(
                    self.n_cores, *self.out_avals[i].shape)[c]
                for i, name in enumerate(self.out_names)
            }
            for c in range(self.n_cores)
        ]
'''

_mod = _types.ModuleType("bass_exec_inline")
exec(compile(_BASS_EXEC_SRC, "bass_exec_inline", "exec"), _mod.__dict__)
_sys.modules["bass_exec_inline"] = _mod



# revision 20
# speedup vs baseline: 3.2700x; 3.2506x over previous
"""DeBut 2D conv (32,128,56,56) -> (32,256,56,56) on 8 axon TRN2 NeuronCores.

The butterfly product W3@W2@W1 composes to a block-diagonal (256,1152) matrix
with 32 blocks of (8 out x 36 in). Finer than the 2-chunk split: output
GROUP g of 32 channels (32g..32g+31, g=0..7) depends only on input features
144g..144g+143, which live inside kernel positions g and g+1. So group g is
exactly TWO kpos-pure K=128 matmuls with weights W[32g:+32, 128(g+j):+128].T
(j=0,1) - the zero rows come for free from block-diagonality.

The PE array is addressed in 128x32 column-tiled mode: 4 col quadrants run
4 concurrent M=32 matmuls, each streaming its own moving span. Per strip
that is 16 matmuls in 4 concurrent rounds = 4x462 PE cycles instead of the
9x462 of the 2-chunk scheme (2.25x less PE stream time), pushing the kernel
from PE-bound to the DMA roofline.

Per-core layout (batch sharded 4 images/core):
  x arrives host-padded to 56x58 (left/right zero cols only), converted to
  bf16 (rel-err budget is 2e-2; full-bf16 pipeline lands ~4e-3), and is
  DMA'd in 4 strip-aligned cuts per image into rows 1..56 of one of 4
  resident SBUF slots; top/bottom pad rows are persistent SBUF zeros.
  bf16 halves both DMA directions; the PE streams 1 col/cycle either way.
  7 strips of 8 output rows; moving operand = contiguous 462-col span of the
  padded image; psum (128, 8, 58) f32; drain cols 0:56 + bias via DVE
  tensor_scalar_add (chunk0) / ACT activation (chunk1), casting to bf16.
  Output pieces are split across BOTH the SWDGE (gpsimd) and ACT HWDGE
  queues; inputs ride the SP HWDGE queue.
  The repeat loop (timing harness) is unrolled 8x inside tc.For_i: each
  For_i iteration ends in a full 5-engine + DMA barrier (~5-8us), so deep
  unroll amortizes it; repeat counts stay exact via trailing bodies.
"""
import numpy as np

# ---- problem constants (hardcoded; kernel.py must be self-contained) ----
B, C_IN, H, W = 32, 128, 56, 56
C_OUT = 256
KS = 3
N_CORES = 8
B_LOC = B // N_CORES          # 4 images per core
HP = H + 2                    # 58
PADDED = HP * HP + 8          # 3372, slack for junk-column overreads
STRIP_ROWS = 8
N_STRIPS = H // STRIP_ROWS    # 7
N_MM = STRIP_ROWS * HP        # 464 moving columns per matmul
N_OUT = STRIP_ROWS * W        # 448 valid columns per strip
R_SHAPES = [(768, 1152, 2, 3, 1), (512, 768, 2, 3, 2), (256, 512, 2, 4, 4)]
# (chunk, kpos) pairs: chunk0 -> kpos 0..4, chunk1 -> kpos 4..8
CHUNK_KPOS = [(0, k) for k in range(5)] + [(1, k) for k in range(4, 9)]

_RUNNER = None


def _compose_w(twiddle: np.ndarray) -> np.ndarray:
    """Compose butterfly factors into the dense (256, 1152) matrix (float64)."""
    W_full = None
    temp = 0
    for (osz, isz, row, col, diag) in R_SHAPES:
        npar = col * osz
        nb = isz // (col * diag)
        t = twiddle[temp:temp + npar].astype(np.float64)
        t = t.reshape(nb, diag, row, col).transpose(0, 2, 3, 1)  # (n, r, c, d)
        temp += npar
        Ws = np.zeros((osz, isz), np.float64)
        # out index n*row*diag + r*diag + d ; in index n*col*diag + c*diag + d
        for d in range(diag):
            for r in range(row):
                for c in range(col):
                    out_idx = np.arange(nb) * row * diag + r * diag + d
                    in_idx = np.arange(nb) * col * diag + c * diag + d
                    Ws[out_idx, in_idx] = t[:, r, c, d]
        W_full = Ws if W_full is None else Ws @ W_full
    return W_full  # (256, 1152)


def _build_nc(repeat: int = 1, trace_sim: bool = False, mode: str = 'full',
              outp_bufs: int = 3, psum_bufs: int = 6, k4merge: bool = True,
              n_xpad: int = 4, drain_split: bool = True, opt: bool = True,
              out_pool: bool = True, early_out: int = 2,
              out_dt: str = 'bf16', out_split2: bool = True,
              in_dt: str = 'bf16', in_cuts: int = 1, unroll: int = 8,
              cuts2: bool = True, tail_sync: bool = False,
              scheme: str = 'grp32', dma4q: bool = False):
    import concourse.bass as bass  # noqa: F401
    from concourse import bacc
    import concourse.mybir as mybir
    from concourse.tile import TileContext

    f32 = mybir.dt.float32
    f32r = mybir.dt.float32r
    bf16 = mybir.dt.bfloat16

    nc = bacc.Bacc("TRN2", target_bir_lowering=False, debug=False,
                   num_devices=N_CORES)
    # xs/wts are declared float32r: same 4-byte layout (numpy float32 binds),
    # lets plain HWDGE DMAs feed the f32r matmuls with no cast pass.
    # xs arrives column-padded to 56x58 so the in-DMA is one contiguous
    # ~13KB run per partition; top/bottom pad rows stay resident zeros.
    idt = bf16 if in_dt == 'bf16' else f32r
    xs = nc.declare_dram_parameter("xs", [B_LOC, C_IN, H * HP], idt,
                                   isOutput=False)
    # grp32: 16 weight mats [128 in-ch, 32 outs], slot 2g+j = group g piece j
    NW, WM = (16, 32) if scheme == 'grp32' else (10, 128)
    wts = nc.declare_dram_parameter("wts", [NW, C_IN, WM], idt, isOutput=False)
    biasT = nc.declare_dram_parameter("biasT", [128, 2], f32, isOutput=False)
    # output stored bf16: halves out-DMA traffic (rel-err budget is 2e-2,
    # bf16 rounding adds ~2e-3); host upcasts to f32
    odt = bf16 if out_dt == 'bf16' else f32
    ys = nc.declare_dram_parameter("ys", [B_LOC, 2, 128, H * W], odt,
                                   isOutput=True)

    K4MERGE = k4merge
    with TileContext(nc, trace_sim=trace_sim) as tc:
        with tc.tile_pool(name="sbuf", bufs=1) as cpool, \
             tc.tile_pool(name="outp", bufs=outp_bufs) as opool, \
             tc.tile_pool(name="psum", bufs=psum_bufs, space="PSUM") as ppool:
            # persistent padded-image slots; 8-col slack zeroed once below
            N_XPAD = n_xpad
            xpads = [cpool.tile([C_IN, PADDED], idt, tag=f"xpad{i}",
                                name=f"xpad{i}")
                     for i in range(N_XPAD)]
            zrow = cpool.tile([C_IN, HP + 8], f32 if in_dt != 'bf16' else bf16,
                              tag="zrow")
            nc.vector.memset(zrow[:], 0.0)
            for xp in xpads:
                nc.vector.tensor_copy(xp[:, :HP], zrow[:, :HP])   # top pad row
                # bottom pad row + slack (never overwritten by image DMAs)
                nc.vector.tensor_copy(xp[:, (HP - 1) * HP:], zrow[:])
            # first image on the SP HWDGE queue, weights+bias on the ACT HWDGE
            # queue - they land in parallel
            # strip s reads image rows 8s-1..8s+9; boundaries (10,26,42)
            # stagger the per-strip waits as c0,c1,c1,c2,c2,c3,c3
            CUTS = ((0, 10 * HP), (10 * HP, 26 * HP), (26 * HP, 42 * HP),
                    (42 * HP, H * HP)) if cuts2 else \
                   ((0, 9 * HP), (9 * HP, 24 * HP), (24 * HP, 40 * HP),
                    (40 * HP, H * HP))
            # dma4q: each HWDGE queue sustains only ~125 GB/s, so balance the
            # 9.85MB/body across four queues: in cuts 0-2 on sync, cut 3 on
            # vector; out pieces spread over gpsimd/scalar/vector below.
            def in_eng(i):
                return nc.vector if (dma4q and i == 3) else nc.sync

            if mode not in ('no_in', 'pe_only', 'dma_out'):
                cuts = CUTS if opt else ((0, 34 * HP), (34 * HP, H * HP))
                for i, (lo, hi) in enumerate(cuts):
                    in_eng(i).dma_start(out=xpads[0][:, HP + lo:HP + hi],
                                        in_=xs[0, :, lo:hi])
            wt_r = cpool.tile([C_IN, NW, WM], idt, tag="wtr")
            # chunk0's slots land first so the first matmul group can start
            # before the full weight transfer completes
            nc.scalar.dma_start(out=wt_r[:, 0:NW // 2, :],
                                in_=wts.ap()[0:NW // 2]
                                .rearrange("i c m -> c i m"))
            nc.scalar.dma_start(out=wt_r[:, NW // 2:NW, :],
                                in_=wts.ap()[NW // 2:NW]
                                .rearrange("i c m -> c i m"))
            bias_sb = cpool.tile([128, 2], f32, tag="bias")
            nc.scalar.dma_start(out=bias_sb[:], in_=biasT.ap())
            scratch = cpool.tile([C_IN, H * HP], idt, tag="scr",
                                 name="scr") \
                if mode == 'scratch_in' else None

            # out-DMA split points; spreading pieces across the image keeps
            # the out queues busy from strip 1 on and shrinks the tail
            if dma4q:
                early_out = 2
            if early_out == 3:
                OUT_SPLITS = {s: (s * N_OUT, (s + 1) * N_OUT)
                              for s in range(N_STRIPS)}
            elif early_out == 2:
                OUT_SPLITS = {1: (0, 2 * N_OUT), 3: (2 * N_OUT, 4 * N_OUT),
                              5: (4 * N_OUT, 6 * N_OUT),
                              6: (6 * N_OUT, 7 * N_OUT)}
            elif early_out:
                OUT_SPLITS = {2: (0, 3 * N_OUT), 4: (3 * N_OUT, 5 * N_OUT),
                              5: (5 * N_OUT, 6 * N_OUT),
                              6: (6 * N_OUT, 7 * N_OUT)}
            else:
                OUT_SPLITS = {3: (0, 4 * N_OUT), 5: (4 * N_OUT, 6 * N_OUT),
                              6: (6 * N_OUT, 7 * N_OUT)}

            def body():
                for b in range(B_LOC):
                    if mode == 'scratch_in' and (b > 0 or repeat > 1):
                        # same DMA traffic, but no PE dependency: writes go to
                        # a scratch tile nobody reads (concurrency probe)
                        nc.sync.dma_start(out=scratch[:], in_=xs[b])
                    elif (b > 0 or repeat > 1) and mode not in ('no_in',
                                                                'pe_only',
                                                                'dma_out'):
                        # in_cuts: 1 = cut every image, 2 = cut only the
                        # first (post-barrier) image — later images hide
                        # under compute, and fewer DMAs means less
                        # completion-semaphore lane aliasing
                        if in_cuts == 1 or (in_cuts == 2 and b == 0):
                            # strip-aligned pieces: first strip's matmuls wait
                            # only the first cut, not the whole image
                            for i, (lo, hi) in enumerate(CUTS):
                                in_eng(i).dma_start(
                                    out=xpads[b % N_XPAD][:, HP + lo:HP + hi],
                                    in_=xs[b, :, lo:hi])
                        else:
                            nc.sync.dma_start(
                                out=xpads[b % N_XPAD][:, HP:HP + H * HP],
                                in_=xs[b])

                    xp = xpads[b % N_XPAD]

                    nmm = N_MM - 2 if opt else N_MM

                    def span(s, k):
                        di, dj = k // 3, k % 3
                        st = (STRIP_ROWS * s + di) * HP + dj
                        return xp[:, st:st + nmm], st

                    out_sb = opool.tile([128, 2, H * W], odt, tag="outsb")
                    if mode in ('dma_only', 'dma_in', 'dma_out'):
                        nc.vector.memset(out_sb[:, :, 0:4], 0.0)
                    for s in range(N_STRIPS):
                        if mode in ('dma_only', 'dma_in', 'dma_out'):
                            pass
                        elif scheme == 'grp32':
                            # 16 col-tiled matmuls (128x32 mode): group g ->
                            # psum quadrant g%4 of chunk g//4; piece j streams
                            # the kpos-(g+j) span. Issue in 4-lane rounds so
                            # the 4 col quadrants run concurrently.
                            ps0 = ppool.tile([128, STRIP_ROWS, HP], f32,
                                             tag="ps")
                            ps1 = ppool.tile([128, STRIP_ROWS, HP], f32,
                                             tag="ps")
                            pss = [ps0, ps1]
                            pfs = [p.rearrange("p r w -> p (r w)") for p in pss]
                            for chunk in range(2):
                                for j in range(2):
                                    for q in range(4):
                                        g = chunk * 4 + q
                                        nc.tensor.matmul(
                                            pfs[chunk][32 * q:32 * q + 32,
                                                       :nmm],
                                            wt_r[:, 2 * g + j, :],
                                            span(s, g + j)[0],
                                            start=(j == 0), stop=(j == 1),
                                            tile_position=(0, 32 * q),
                                            skip_group_check=True)
                        elif not K4MERGE:
                            pss = []
                            for chunk in range(2):
                                ps = ppool.tile([128, STRIP_ROWS, HP], f32,
                                                tag="ps")
                                pss.append(ps)
                                psflat = ps.rearrange("p r w -> p (r w)")
                                for kidx in range(5):
                                    _, k = CHUNK_KPOS[chunk * 5 + kidx]
                                    nc.tensor.matmul(
                                        psflat[:, :nmm],
                                        wt_r[:, chunk * 5 + kidx, :],
                                        span(s, k)[0],
                                        start=(kidx == 0), stop=(kidx == 4))
                        else:
                            # chunk0 <- k0..3, chunk1 <- k5..8 (full-array MMs),
                            # then the two half-K kpos-4 MMs run concurrently in
                            # disjoint (row_grp, col_grp) array tiles.
                            ps0 = ppool.tile([128, STRIP_ROWS, HP], f32, tag="ps")
                            ps1 = ppool.tile([128, STRIP_ROWS, HP], f32, tag="ps")
                            pss = [ps0, ps1]
                            pf0 = ps0.rearrange("p r w -> p (r w)")
                            pf1 = ps1.rearrange("p r w -> p (r w)")
                            for kidx, k in enumerate((0, 1, 2, 3)):
                                nc.tensor.matmul(
                                    pf0[:, :nmm], wt_r[:, k, :], span(s, k)[0],
                                    start=(kidx == 0), stop=False)
                            for kidx, k in enumerate((5, 6, 7, 8)):
                                nc.tensor.matmul(
                                    pf1[:, :nmm], wt_r[:, k, :], span(s, k)[0],
                                    start=(kidx == 0), stop=False)
                            _, st4 = span(s, 4)
                            nc.tensor.matmul(
                                pf0[:, :nmm], wt_r[0:64, 4, :],
                                xp[0:64, st4:st4 + nmm],
                                start=False, stop=True,
                                tile_position=(0, 0), skip_group_check=True)
                            nc.tensor.matmul(
                                pf1[:, :nmm], wt_r[64:128, 4, :],
                                xp[64:128, st4:st4 + nmm],
                                start=False, stop=True,
                                tile_position=(64, 0), skip_group_check=True)
                        for chunk in range(2) if mode not in ('dma_only', 'dma_in', 'dma_out') else ():
                            dst = out_sb[:, chunk, s * N_OUT:(s + 1) * N_OUT] \
                                .rearrange("p (r w) -> p r w", w=W)
                            if drain_split and chunk == 1:
                                nc.scalar.activation(
                                    dst, pss[chunk][:, :, 0:W],
                                    mybir.ActivationFunctionType.Identity,
                                    bias=bias_sb[:, chunk:chunk + 1], scale=1.0)
                            else:
                                nc.vector.tensor_scalar_add(
                                    dst, pss[chunk][:, :, 0:W],
                                    bias_sb[:, chunk:chunk + 1],
                                )
                        if s in OUT_SPLITS:
                            lo, hi = OUT_SPLITS[s]
                            if mode in ('no_out', 'pe_only', 'dma_in'):
                                if s == 6:
                                    nc.scalar.dma_start(out=ys[b, 0, :, :16],
                                                        in_=out_sb[:, 0, :16])
                            elif dma4q:
                                # per image: s1 -> gpsimd, s3 -> scalar,
                                # s5 -> half gpsimd + half scalar, s6 -> vector
                                # (vector also carries in-cut 3) ~0.6MB/queue
                                if s == 5:
                                    mid = (lo + hi) // 2
                                    plan = ((nc.gpsimd, lo, mid),
                                            (nc.scalar, mid, hi))
                                else:
                                    e = {1: nc.gpsimd, 3: nc.scalar,
                                         6: nc.vector}[s]
                                    plan = ((e, lo, hi),)
                                for e, l2, h2 in plan:
                                    e.dma_start(
                                        out=ys[b, :, :, l2:h2]
                                        .rearrange("c2 p hw -> p c2 hw"),
                                        in_=out_sb[:, :, l2:h2])
                            else:
                                eng = nc.gpsimd if (out_pool or
                                                    (opt and (b + s) % 2)) \
                                    else nc.scalar
                                if out_split2:
                                    # every piece: halves on two queues; the
                                    # final piece optionally rides the two
                                    # low-latency HWDGE rings (sync is idle
                                    # at image end) to shrink the tail
                                    mid = (lo + hi) // 2
                                    e1 = nc.sync if (tail_sync and
                                                     s == N_STRIPS - 1) \
                                        else nc.gpsimd
                                    for e, l2, h2 in ((e1, lo, mid),
                                                      (nc.scalar, mid, hi)):
                                        e.dma_start(
                                            out=ys[b, :, :, l2:h2]
                                            .rearrange("c2 p hw -> p c2 hw"),
                                            in_=out_sb[:, :, l2:h2])
                                elif opt and s == N_STRIPS - 1:
                                    # final piece: halves on two queues to
                                    # shrink the kernel tail
                                    mid = (lo + hi) // 2
                                    for e, l2, h2 in ((nc.gpsimd, lo, mid),
                                                      (nc.scalar, mid, hi)):
                                        e.dma_start(
                                            out=ys[b, :, :, l2:h2]
                                            .rearrange("c2 p hw -> p c2 hw"),
                                            in_=out_sb[:, :, l2:h2])
                                else:
                                    eng.dma_start(
                                        out=ys[b, :, :, lo:hi]
                                        .rearrange("c2 p hw -> p c2 hw"),
                                        in_=out_sb[:, :, lo:hi])

            if repeat == 1:
                body()
            else:
                n_loop = repeat // unroll
                if n_loop > 0:
                    with tc.For_i(0, n_loop, 1,
                                  hint_engines=(mybir.EngineType.PE,)):
                        for _ in range(unroll):
                            body()
                for _ in range(repeat - n_loop * unroll):
                    body()
    nc.finalize()
    return nc


def _get_runner(repeat: int = 1):
    global _RUNNER
    if _RUNNER is None or _RUNNER[0] != repeat:
        from bass_exec_inline import BassRunner
        nc = _build_nc(repeat)
        _RUNNER = (repeat, BassRunner(nc, n_cores=N_CORES))
    return _RUNNER[1]


def _prep_params(twiddle: np.ndarray, bias: np.ndarray, k4merge: bool = True,
                 in_dt: str = 'bf16', scheme: str = 'grp32'):
    W_dense = _compose_w(np.asarray(twiddle))
    if scheme == 'grp32':
        # group g (outs 32g..32g+31) = sum_j W[32g:+32, 128(g+j):+128] @ x_kpos
        # block-diagonality makes rows outside [144g, 144g+144) exactly zero
        wts = np.zeros((16, C_IN, 32), np.float32)
        for g in range(8):
            for j in range(2):
                c0 = 128 * (g + j)
                wts[2 * g + j] = W_dense[32 * g:32 * g + 32,
                                         c0:c0 + 128].T.astype(np.float32)
        biasT = np.asarray(bias, np.float32).reshape(2, 128).T.copy()
        if in_dt == 'bf16':
            import ml_dtypes
            wts = wts.astype(ml_dtypes.bfloat16)
        return wts, biasT
    wts = np.zeros((10, C_IN, 128), np.float32)
    if k4merge:
        # slot k (k != 4): full W slice for kpos k into its chunk
        for k in range(9):
            if k == 4:
                continue
            chunk = 0 if k < 4 else 1
            blk = W_dense[chunk * 128:(chunk + 1) * 128, 128 * k:128 * (k + 1)]
            wts[k] = blk.T.astype(np.float32)
        # slot 4 packed for the row-tiled pair: ch 0..63 carry chunk0's kpos-4
        # weights (full co 0..127), ch 64..127 carry chunk1's (co 128..255)
        wts[4][0:64, :] = W_dense[0:128, 512:576].T.astype(np.float32)
        wts[4][64:128, :] = W_dense[128:256, 576:640].T.astype(np.float32)
    else:
        for i, (chunk, k) in enumerate(CHUNK_KPOS):
            blk = W_dense[chunk * 128:(chunk + 1) * 128, 128 * k:128 * (k + 1)]
            wts[i] = blk.T.astype(np.float32)
    biasT = np.asarray(bias, np.float32).reshape(2, 128).T.copy()
    if in_dt == 'bf16':
        import ml_dtypes
        wts = wts.astype(ml_dtypes.bfloat16)
    return wts, biasT


def _prep_x(x: np.ndarray, in_dt: str = 'bf16') -> np.ndarray:
    """(32,128,56,56) -> column-padded (8, 4, 128, 56*58); the top/bottom
    pad rows live as persistent zeros in SBUF (never transferred)."""
    x = np.asarray(x, np.float32).reshape(B, C_IN, H, W)
    dt = np.float32
    if in_dt == 'bf16':
        import ml_dtypes
        dt = ml_dtypes.bfloat16
    xp = np.zeros((B, C_IN, H, HP), dt)
    xp[:, :, :, 1:1 + W] = x.astype(dt)
    return xp.reshape(N_CORES, B_LOC, C_IN, H * HP)


def kernel(x: np.ndarray, twiddle: np.ndarray, bias: np.ndarray) -> np.ndarray:
    wts, biasT = _prep_params(twiddle, bias)
    runner = _get_runner(1)
    xsh = _prep_x(x)
    in_maps = [{"xs": xsh[c], "wts": wts, "biasT": biasT} for c in range(N_CORES)]
    res = runner(runner.pack(in_maps))
    out = np.stack([res[c]["ys"] for c in range(N_CORES)])  # (8,4,2,128,3136)
    return out.reshape(B, C_OUT, H, W).astype(np.float32)


# ---- inline copy of the reusable jitted runner (kernel.py self-contained) --
import sys as _sys
import types as _types

_BASS_EXEC_SRC = '''
import numpy as np
import jax
from jax.sharding import Mesh, PartitionSpec
from jax.experimental.shard_map import shard_map

import concourse.mybir as mybir
from concourse.bass2jax import _bass_exec_p, partition_id_tensor, install_neuronx_cc_hook


class BassRunner:
    def __init__(self, nc, n_cores=8):
        install_neuronx_cc_hook()
        assert nc.is_finalized()
        self.nc = nc
        self.n_cores = n_cores
        partition_name = nc.partition_id_tensor.name if nc.partition_id_tensor else None

        in_names, out_names, out_avals, zero_outs = [], [], [], []
        for alloc in nc.m.functions[0].allocations:
            if not isinstance(alloc, mybir.MemoryLocationSet):
                continue
            name = alloc.memorylocations[0].name
            if alloc.kind == "ExternalInput":
                if name != partition_name:
                    in_names.append(name)
            elif alloc.kind == "ExternalOutput":
                out_names.append(name)
                shape = tuple(alloc.tensor_shape)
                dtype = mybir.dt.np(alloc.dtype)
                out_avals.append(jax.core.ShapedArray(shape, dtype))
                zero_outs.append(np.zeros(shape, dtype))
        self.n_params = len(in_names)
        n_outs = len(out_avals)
        self.in_names = list(in_names)
        self.out_names = out_names
        self.out_avals = out_avals
        self.zero_outs = zero_outs
        all_in_names = in_names + out_names
        if partition_name is not None:
            all_in_names.append(partition_name)

        donate = tuple(range(self.n_params, self.n_params + n_outs))

        def _body(*args):
            operands = list(args)
            if partition_name is not None:
                operands.append(partition_id_tensor())
            outs = _bass_exec_p.bind(
                *operands,
                out_avals=tuple(out_avals),
                in_names=tuple(all_in_names),
                out_names=tuple(out_names),
                lowering_input_output_aliases=(),
                sim_require_finite=True,
                sim_require_nnan=True,
                nc=nc,
            )
            return tuple(outs)

        devices = jax.devices()[:n_cores]
        mesh = Mesh(np.asarray(devices), ("core",))
        self._mesh = mesh
        self._zeros_fn = None
        in_specs = (PartitionSpec("core"),) * (self.n_params + n_outs)
        out_specs = (PartitionSpec("core"),) * len(out_names)
        self._fn = jax.jit(
            shard_map(_body, mesh=mesh, in_specs=in_specs, out_specs=out_specs,
                      check_rep=False),
            donate_argnums=donate, keep_unused=True,
        )


    def pack_device(self, in_maps):
        """device_put the packed inputs once; reuse across calls."""
        import jax.numpy as jnp
        from jax.sharding import NamedSharding
        concat = self.pack(in_maps)
        sh = NamedSharding(self._mesh, PartitionSpec("core"))
        return [jax.device_put(a, sh) for a in concat]

    def zeros_device(self):
        if self._zeros_fn is None:
            import jax.numpy as jnp
            from jax.sharding import NamedSharding
            sh = NamedSharding(self._mesh, PartitionSpec("core"))
            shapes = [(self.n_cores * z.shape[0], *z.shape[1:]) for z in self.zero_outs]
            dts = [z.dtype for z in self.zero_outs]

            def _mk():
                return tuple(jnp.zeros(s, d) for s, d in zip(shapes, dts))
            self._zeros_fn = jax.jit(_mk, out_shardings=tuple([sh] * len(shapes)))
        return self._zeros_fn()

    def call_device(self, concat_in_dev):
        """Device-resident call: returns raw jax output arrays."""
        zeros = self.zeros_device()
        return self._fn(*concat_in_dev, *zeros)

    def pack(self, in_maps):
        per_core = [[np.asarray(m[name]) for name in self.in_names] for m in in_maps]
        return [
            np.concatenate([per_core[c][i] for c in range(self.n_cores)], axis=0)
            for i in range(self.n_params)
        ]

    def __call__(self, concat_in, raw=False):
        concat_zeros = [
            np.zeros((self.n_cores * z.shape[0], *z.shape[1:]), z.dtype)
            for z in self.zero_outs
        ]
        out_arrs = self._fn(*concat_in, *concat_zeros)
        if raw:
            return out_arrs
        return [
            {
                name: np.asarray(out_arrs[i]).reshape(
                    self.n_cores, *self.out_avals[i].shape)[c]
                for i, name in enumerate(self.out_names)
            }
            for c in range(self.n_cores)
        ]
'''

_mod = _types.ModuleType("bass_exec_inline")
exec(compile(_BASS_EXEC_SRC, "bass_exec_inline", "exec"), _mod.__dict__)
_sys.modules["bass_exec_inline"] = _mod

